# revision 11
# baseline (speedup 1.0000x reference)
"""DigitCapsule (dynamic routing) Trainium2 Bass kernel.

Problem: x (128,1152,8) f32, W (1,1152,10,16,8) f32 ->
  u_hat[b,r,o,do] = sum_di W[r,o,do,di] x[b,r,di]
  3 routing iterations (softmax over routes r, elementwise squash),
  output v (128,10,16,1).

Sharding: data-parallel over batch, 16 samples per core, W replicated.

Per-core layout (partition p = 16*j + b, j = r mod 8, b = batch-in-core):
  u[p, cc, do, o] = u_hat[b, 8*cc+j, o, do]   (fp16, 144 x 16 x 10 free)
u_hat is produced by 144 matmuls with a block-diagonal stationary operand
X_diag[(j,di)=64, (j',b)=128] (8 routes x 8 di contracted per matmul).
Route-sums (softmax denominator, s_j) are matmuls with a constant
delta matrix D[p,q] = (p%16 == q%16) that sums the 8 j-lanes per b and
replicates the result across all 128 partitions. The do-contraction
(agreement) is a pairwise fp16 adder tree split across DVE and GPSIMD.

Engine budget (cost-model): DVE runs fp16 2-ops at 0.52 ns/elem and
fp16 copies at 0.26; Act copies at 0.833; GPSIMD adds at 1.98. PE matmul
costs out-cols x pe_cycle with free ldweights, so the d16 folds ride on
otherwise-idle PE time. Zero-fill and constant generation avoid the
1.04 ns/elem Memset and the HWDGE fixed cost of extra DMAs.
"""

import numpy as np

import concourse.bacc as bacc
import concourse.bass as bass
import concourse.tile as tile
from concourse import mybir
from concourse.bass_utils import run_bass_kernel_spmd

B, R, O, DO, DI = 128, 1152, 10, 16, 8
NCORES = 8
BC = B // NCORES          # 16 samples per core
J = 8                     # routes per matmul group
CC = R // J               # 144 matmul groups
OD = O * DO               # 160
F16 = mybir.dt.float16
F32 = mybir.dt.float32

PROD_BATCH = 3            # cc per production psum batch (1 bank each)
TREE_BATCH = 24           # cc per premult/tree batch
PROD_LAG = 2              # psum batches between produce and s0-consume


def _squash_chain(nc, pool, s_ps, v_out, scale, eps):
    """v_out = squash(s_ps * scale); s_ps is [P,16,10] f32 (PSUM).

    squash here is elementwise (reference reduces over a singleton axis):
    v = s*m/((1+m)*sqrt(m+eps)), m = s^2.
    """
    P = s_ps.shape[0]
    s_sb = pool.tile([P, DO, O], F32, tag="sq_s")
    m = pool.tile([P, DO, O], F32, tag="sq_m")
    r = pool.tile([P, DO, O], F32, tag="sq_r")
    d = pool.tile([P, DO, O], F32, tag="sq_d")
    p1 = pool.tile([P, DO, O], F32, tag="sq_p1")
    y = pool.tile([P, DO, O], F32, tag="sq_y")
    nc.scalar.mul(s_sb[:], s_ps[:], scale)
    nc.scalar.square(m[:], s_sb[:])
    nc.scalar.activation(r[:], m[:], mybir.ActivationFunctionType.Sqrt, bias=eps[:P])
    nc.vector.reciprocal(y[:], r[:])                # y0 ~ rsqrt(m+eps), ~0.4% err
    # one Newton step: y1 = y0*(1.5 - 0.5*m*y0^2)
    nc.vector.tensor_mul(r[:], m[:], y[:])
    nc.vector.tensor_mul(r[:], r[:], y[:])
    nc.vector.tensor_scalar(r[:], r[:], -0.5, 1.5, mybir.AluOpType.mult,
                            mybir.AluOpType.add)
    nc.vector.tensor_mul(y[:], y[:], r[:])
    nc.vector.tensor_scalar_add(d[:], m[:], 1.0)
    nc.vector.reciprocal(d[:], d[:])
    nc.vector.tensor_mul(p1[:], s_sb[:], m[:])      # s*m
    nc.vector.tensor_mul(p1[:], p1[:], y[:])        # s*m*rsqrt
    nc.vector.tensor_mul(v_out[:], p1[:], d[:])     # -> v (cast on write)


def build_nc(reps=1):
    nc = bacc.Bacc("TRN2", debug=False)
    wt_d = nc.dram_tensor("wt", [64, CC, DO, O], F16, kind="ExternalInput")
    # xf[j, di, b, cc] = x[b, 8cc+j, di], compact (no zeros)
    xf_d = nc.dram_tensor("xf", [J, DI, BC, CC], F16, kind="ExternalInput")
    dout_d = nc.dram_tensor("dout", [128, BC], F16, kind="ExternalInput")
    out_d = nc.dram_tensor("out", [BC, O, DO], F32, kind="ExternalOutput")

    NB = CC // PROD_BATCH     # 48 production batches
    NG = CC // TREE_BATCH     # 6 tree batches

    with tile.TileContext(nc) as tc:
        with (
            tc.tile_pool(name="const", bufs=1) as const,
            tc.tile_pool(name="prod", bufs=1) as prod,
            tc.tile_pool(name="main", bufs=1) as main,
            tc.tile_pool(name="sq", bufs=2) as sq,
            tc.tile_pool(name="tp", bufs=2) as tp,
            tc.tile_pool(name="l1p", bufs=2) as l1p,
            tc.tile_pool(name="l2p", bufs=2) as l2p,
            tc.tile_pool(name="l3p", bufs=2) as l3p,
            tc.tile_pool(name="l4p", bufs=2) as l4p,
            tc.tile_pool(name="pp", bufs=2, space=bass.MemorySpace.PSUM) as pp,
            tc.tile_pool(name="pss", bufs=1, space=bass.MemorySpace.PSUM) as pss,
            tc.tile_pool(name="psd", bufs=1, space=bass.MemorySpace.PSUM) as psd,
        ):
            eps = const.tile([128, 1], F32)
            zero = const.tile([128, 1], F32)
            nc.vector.memset(eps[:], 1e-9)
            nc.vector.memset(zero[:], 0.0)
            # preload activation tables while DMAs are in flight
            warm = const.tile([128, 1], F32)
            nc.scalar.copy(warm[:], zero[:])
            nc.scalar.square(warm[:], zero[:])
            nc.scalar.activation(warm[:], zero[:],
                                 mybir.ActivationFunctionType.Sqrt, bias=eps[:])
            nc.scalar.activation(warm[:], zero[:],
                                 mybir.ActivationFunctionType.Exp, bias=zero[:])
            nc.scalar.add(warm[:], zero[:], 0.0)
            dout = const.tile([128, BC], F16)
            nc.sync.dma_start(dout[:], dout_d[:])
            # d16[p,q] = (p%16 == q%16) built on-chip: 8 tiled copies of dout
            d16 = const.tile([128, 128], F16)
            d32 = const.tile([128, 128], F32)
            nc.vector.tensor_copy(
                d16[:].rearrange("p (j b) -> p j b", j=J),
                dout[:].unsqueeze(1).broadcast_to((128, J, BC)))
            nc.vector.tensor_copy(d32[:], d16[:])

            for _rep in range(reps):
                # ---- W chunks first so they don't queue behind the
                # scatters' zero-fill semaphore wait on the SP queue ----
                wt = prod.tile([64, CC, DO, O], F16)
                NW = 2
                wchunk = CC // NW
                for w in range(NW):
                    sl = slice(w * wchunk, (w + 1) * wchunk)
                    nc.sync.dma_start(wt[:, sl], wt_d[:, sl])

                # ---- x block-diagonal staging (fast zero-fill + scatter) ----
                xd = prod.tile([64, 128, CC], F16)   # [k=(j,di), col=(j,b), cc]
                zsplit = 84
                zstrip = const.tile([64, CC], F16)
                nc.vector.memset(zstrip[:], 0.0)
                nc.vector.tensor_copy(
                    xd[:, 0:zsplit, :],
                    zstrip[:].unsqueeze(1).broadcast_to((64, zsplit, CC)))
                xz = xd[:, zsplit:128, :].bitcast(mybir.dt.uint32)
                nc.scalar.mul(xz, xz, 0.0)
                for j in range(J):
                    nc.sync.dma_start(
                        xd[8 * j: 8 * j + 8, 16 * j: 16 * j + 16, :],
                        xf_d[j])

                u = main.tile([128, CC, DO, O], F16)

                # ---- produce u_hat; interleave s0 fold on PE; copies on Act/DVE
                s0 = pss.tile([128, DO, O], F32, tag="s")
                for g in range(NB + PROD_LAG):
                    if g < NB:
                        ps = pp.tile([128, PROD_BATCH, 512], F32, tag="pp")
                        for i in range(PROD_BATCH):
                            cc = g * PROD_BATCH + i
                            nc.tensor.matmul(
                                ps[:, i, 0:OD], xd[:, :, cc], wt[:, cc, :, :],
                                start=True, stop=True,
                            )
                        sl = slice(g * PROD_BATCH, (g + 1) * PROD_BATCH)
                        src = ps[:, :, 0:OD].rearrange(
                            "p c (do o) -> p c do o", do=DO)
                        # alternate copies Act/DVE (GPSIMD cannot read PSUM)
                        if g % 2 == 0:
                            nc.scalar.copy(u[:, sl, :, :], src)
                        else:
                            nc.vector.tensor_copy(u[:, sl, :, :], src)
                    if g >= PROD_LAG:
                        gs = g - PROD_LAG
                        for i in range(PROD_BATCH):
                            cc = gs * PROD_BATCH + i
                            nc.tensor.matmul(
                                s0[:], d16[:],
                                u[:, cc, :, :],
                                start=(cc == 0), stop=(cc == CC - 1),
                            )

                v = main.tile([128, DO, O], F16)
                _squash_chain(nc, sq, s0, v, 1.0 / R, eps)

                b16 = main.tile([128, CC, O], F16)
                e = main.tile([128, CC, O], F32)
                inv = main.tile([128, O], F32)
                c16 = main.tile([128, CC, O], F16)

                for it in (1, 2):
                    final = it == 2
                    den = psd.tile([128, O], F32, tag="den")
                    # ---- agreement: b_ij (+)= sum_do u * v  (tree) + softmax num
                    for g in range(NG):
                        sl = slice(g * TREE_BATCH, (g + 1) * TREE_BATCH)
                        t = tp.tile([128, TREE_BATCH, DO, O], F16, tag="t")
                        v_b = v[:].unsqueeze(1).broadcast_to(
                            (128, TREE_BATCH, DO, O))
                        nc.vector.tensor_mul(t[:], u[:, sl, :, :], v_b)
                        l1 = l1p.tile([128, TREE_BATCH, 8, O], F16, tag="l1")
                        nc.vector.tensor_add(
                            l1[:], t[:, :, 0:8, :], t[:, :, 8:16, :])
                        l2 = l2p.tile([128, TREE_BATCH, 4, O], F16, tag="l2")
                        nc.vector.tensor_add(
                            l2[:], l1[:, :, 0:4, :], l1[:, :, 4:8, :])
                        # lower tree levels on GPSIMD to free DVE
                        l3 = l3p.tile([128, TREE_BATCH, 2, O], F16, tag="l3")
                        nc.gpsimd.tensor_add(
                            l3[:], l2[:, :, 0:2, :], l2[:, :, 2:4, :])
                        if it == 1:
                            nc.gpsimd.tensor_add(
                                b16[:, sl, :], l3[:, :, 0, :], l3[:, :, 1, :])
                        else:
                            l4 = l4p.tile([128, TREE_BATCH, O], F16, tag="l4")
                            nc.gpsimd.tensor_add(
                                l4[:], l3[:, :, 0, :], l3[:, :, 1, :])
                            nc.gpsimd.tensor_add(
                                b16[:, sl, :], b16[:, sl, :], l4[:])
                        # exp in f32 (no max subtraction needed in f32)
                        nc.scalar.activation(
                            e[:, sl, :], b16[:, sl, :],
                            mybir.ActivationFunctionType.Exp, bias=zero[:])
                        # softmax denominator: fold routes on PE (idle here)
                        for i in range(TREE_BATCH):
                            cc = g * TREE_BATCH + i
                            nc.tensor.matmul(
                                den[:], d32[:], e[:, cc, :],
                                start=(cc == 0), stop=(cc == CC - 1),
                            )

                    nc.vector.reciprocal(inv[:], den[:])

                    # ---- s = sum_r c*u: c premult (DVE) + d16 fold (PE) ----
                    sp_p = BC if final else 128
                    lhs = dout[:] if final else d16[:]
                    s_ps2 = pss.tile([sp_p, DO, O], F32, tag="s")
                    for g in range(NG):
                        sl = slice(g * TREE_BATCH, (g + 1) * TREE_BATCH)
                        inv_b = inv[:].unsqueeze(1).broadcast_to(
                            (128, TREE_BATCH, O))
                        nc.vector.tensor_mul(c16[:, sl, :], e[:, sl, :], inv_b)
                        t = tp.tile([128, TREE_BATCH, DO, O], F16, tag="t")
                        c_b = c16[:, sl, :].unsqueeze(2).broadcast_to(
                            (128, TREE_BATCH, DO, O))
                        nc.vector.tensor_mul(t[:], u[:, sl, :, :], c_b)
                        for i in range(TREE_BATCH):
                            cc = g * TREE_BATCH + i
                            nc.tensor.matmul(
                                s_ps2[:], lhs[:, :sp_p], t[:, i, :, :],
                                start=(cc == 0), stop=(cc == CC - 1),
                            )
                    if not final:
                        _squash_chain(nc, sq, s_ps2, v, 1.0, eps)
                    else:
                        v2 = main.tile([BC, DO, O], F32)
                        _squash_chain(nc, sq, s_ps2, v2, 1.0, eps)
                        v2p = main.tile([BC, O, DO], F32)
                        nc.vector.tensor_copy(v2p[:], v2[:].transpose((0, 2, 1)))
                        nc.sync.dma_start(out_d[:], v2p[:])

    nc.compile()
    return nc


_CACHE = {}


def _get_nc():
    if "nc" not in _CACHE:
        _CACHE["nc"] = build_nc()
    return _CACHE["nc"]


def _prep_const():
    if "const" not in _CACHE:
        p = np.arange(128)
        dout = (p[:, None] % 16 == np.arange(BC)[None, :]).astype(np.float16)
        _CACHE["const"] = dout
    return _CACHE["const"]


def kernel(x: np.ndarray, W: np.ndarray) -> np.ndarray:
    x = np.asarray(x, dtype=np.float32)
    W = np.asarray(W, dtype=np.float32)
    nc = _get_nc()
    dout = _prep_const()
    W5 = np.ascontiguousarray(W.reshape(R, O, DO, DI))
    # wt[8j+di, cc, do, o] = W[8cc+j, o, do, di]
    wt = np.ascontiguousarray(
        W5.reshape(CC, J, O, DO, DI).transpose(1, 4, 0, 3, 2)
    ).reshape(64, CC, DO, O).astype(np.float16)
    in_maps = []
    for q in range(NCORES):
        xq = x[BC * q: BC * (q + 1)]             # [16, 1152, 8]
        # xf[j, di, b, cc] = xq[b, 8cc+j, di]
        xf = np.ascontiguousarray(
            xq.reshape(BC, CC, J, DI).transpose(2, 3, 0, 1)
        ).astype(np.float16)
        in_maps.append({"wt": wt, "xf": xf, "dout": dout})
    res = run_bass_kernel_spmd(nc, in_maps, core_ids=list(range(NCORES)))
    out = np.concatenate([res.results[q]["out"] for q in range(NCORES)], axis=0)
    return out.reshape(B, O, DO, 1).astype(np.float32)


# revision 13
# speedup vs baseline: 1.0835x; 1.0835x over previous
"""DigitCapsule (dynamic routing) Trainium2 Bass kernel.

Problem: x (128,1152,8) f32, W (1,1152,10,16,8) f32 ->
  u_hat[b,r,o,do] = sum_di W[r,o,do,di] x[b,r,di]
  3 routing iterations (softmax over routes r, elementwise squash),
  output v (128,10,16,1).

Sharding: data-parallel over batch, 16 samples per core, W replicated.

Per-core layout (partition p = 16*j + b, j = r mod 8, b = batch-in-core):
  u[p, cc, do, o] = u_hat[b, 8*cc+j, o, do]   (fp16, 144 x 16 x 10 free)
u_hat is produced by 144 matmuls with a block-diagonal stationary operand
X_diag[(j,di)=64, (j',b)=128] (8 routes x 8 di contracted per matmul).
Route-sums (softmax denominator, s_j) are matmuls with a constant
delta matrix D[p,q] = (p%16 == q%16) that sums the 8 j-lanes per b and
replicates the result across all 128 partitions. The do-contraction
(agreement) is a pairwise fp16 adder tree split across DVE and GPSIMD.

Perf notes (cost-model driven):
- PE matmul costs out-cols x pe_cycle with FREE ldweights; pe_cycle ramps
  0.83 -> 0.4167 ns only after ~3us of continuous execution, so dummy
  matmuls keep PE hot across DVE-bound phases.
- DVE fp16 2-operand ops: 0.52 ns/elem; fp16 copies 0.26; reductions and
  anything touching f32/PSUM: 1.04. Act 0.833 (copies/exp). GPSIMD adds
  1.98 but runs in parallel.
- Zero-fill via broadcast tensor_copy (4x mode) + Act memzero, not Memset.
"""

import numpy as np

import concourse.bacc as bacc
import concourse.bass as bass
import concourse.tile as tile
from concourse import mybir
from concourse.bass_utils import run_bass_kernel_spmd

B, R, O, DO, DI = 128, 1152, 10, 16, 8
NCORES = 8
BC = B // NCORES          # 16 samples per core
J = 8                     # routes per matmul group
CC = R // J               # 144 matmul groups
OD = O * DO               # 160
F16 = mybir.dt.float16
F32 = mybir.dt.float32

PROD_BATCH = 3            # cc per production psum batch (1 bank each)
TREE_BATCH = 24           # cc per premult/tree batch
PROD_LAG = 2              # psum batches between produce and s0-consume


def _squash_chain(nc, pool, s_ps, v_out, eps):
    """v_out = squash(s_ps); s_ps is [P,16,10] f32 (PSUM).

    squash is elementwise (reference reduces over a singleton axis):
    v = s*m/((1+m)*sqrt(m+eps)), m = s^2. Minimal serial chain:
      m = s*s (DVE), r = sqrt(m+eps) (Act),
      t2 = (m+1)*r (DVE stt), rec = 1/t2 (DVE),
      p = s*m (DVE), v = p*rec (DVE).
    """
    P = s_ps.shape[0]
    s_sb = pool.tile([P, DO, O], F32, tag="sq_s")
    m = pool.tile([P, DO, O], F32, tag="sq_m")
    r = pool.tile([P, DO, O], F32, tag="sq_r")
    t2 = pool.tile([P, DO, O], F32, tag="sq_t2")
    rec = pool.tile([P, DO, O], F32, tag="sq_rec")
    p1 = pool.tile([P, DO, O], F32, tag="sq_p1")
    nc.vector.tensor_copy(s_sb[:], s_ps[:])
    nc.vector.tensor_mul(m[:], s_sb[:], s_sb[:])
    nc.scalar.activation(r[:], m[:], mybir.ActivationFunctionType.Sqrt,
                         bias=eps[:P])
    nc.vector.tensor_mul(p1[:], s_sb[:], m[:])
    nc.vector.scalar_tensor_tensor(t2[:], m[:], 1.0, r[:],
                                   mybir.AluOpType.add, mybir.AluOpType.mult)
    nc.vector.reciprocal(rec[:], t2[:])
    nc.vector.tensor_mul(v_out[:], p1[:], rec[:])


def build_nc(reps=1):
    nc = bacc.Bacc("TRN2", debug=False)
    wt_d = nc.dram_tensor("wt", [64, CC, DO, O], F16, kind="ExternalInput")
    # xf[j, di, b, cc] = x[b, 8cc+j, di], compact (no zeros)
    xf_d = nc.dram_tensor("xf", [J, DI, BC, CC], F16, kind="ExternalInput")
    dout_d = nc.dram_tensor("dout", [128, BC], F16, kind="ExternalInput")
    out_d = nc.dram_tensor("out", [BC, O, DO], F32, kind="ExternalOutput")

    NB = CC // PROD_BATCH     # 48 production batches
    NG = CC // TREE_BATCH     # 6 tree batches

    with tile.TileContext(nc) as tc:
        with (
            tc.tile_pool(name="const", bufs=1) as const,
            tc.tile_pool(name="prod", bufs=1) as prod,
            tc.tile_pool(name="main", bufs=1) as main,
            tc.tile_pool(name="sq", bufs=2) as sq,
            tc.tile_pool(name="tp", bufs=2) as tp,
            tc.tile_pool(name="l1p", bufs=2) as l1p,
            tc.tile_pool(name="l2p", bufs=2) as l2p,
            tc.tile_pool(name="l3p", bufs=2) as l3p,
            tc.tile_pool(name="l4p", bufs=2) as l4p,
            tc.tile_pool(name="pp", bufs=2, space=bass.MemorySpace.PSUM) as pp,
            tc.tile_pool(name="pss", bufs=1, space=bass.MemorySpace.PSUM) as pss,
            tc.tile_pool(name="psd", bufs=1, space=bass.MemorySpace.PSUM) as psd,
        ):
            eps = const.tile([128, 1], F32)
            zero = const.tile([128, 1], F32)
            nc.vector.memset(eps[:], 1e-9)
            nc.vector.memset(zero[:], 0.0)
            # preload activation tables while DMAs are in flight
            warm = const.tile([128, 1], F32)
            nc.scalar.copy(warm[:], zero[:])
            nc.scalar.activation(warm[:], zero[:],
                                 mybir.ActivationFunctionType.Sqrt, bias=eps[:])
            nc.scalar.activation(warm[:], zero[:],
                                 mybir.ActivationFunctionType.Exp, bias=zero[:])

            dout = const.tile([128, BC], F16)
            nc.sync.dma_start(dout[:], dout_d[:])

            # W first so it is not queued behind the scatters' zero-fill wait
            wt = prod.tile([64, CC, DO, O], F16)
            NW = 2
            wchunk = CC // NW
            for w in range(NW):
                slw = slice(w * wchunk, (w + 1) * wchunk)
                nc.sync.dma_start(wt[:, slw], wt_d[:, slw])

            # d16[p,q] = (p%16 == q%16); d16s = d16/R (iter-0 softmax fold)
            d16 = const.tile([128, 128], F16)
            d16s = const.tile([128, 128], F16)
            d32 = const.tile([128, 128], F32)
            nc.vector.tensor_copy(
                d16[:].rearrange("p (j b) -> p j b", j=J),
                dout[:].unsqueeze(1).broadcast_to((128, J, BC)))
            nc.vector.tensor_scalar_mul(d16s[:], d16[:], 1.0 / R)
            nc.vector.tensor_copy(d32[:], d16[:])

            # PE warm-up: dummy matmuls ramp the p-state before production
            zstrip = const.tile([64, CC], F16)
            nc.vector.memset(zstrip[:], 0.0)
            dummy_ps = psd.tile([128, 512], F32, tag="den")

            def pe_warm(n):
                for _ in range(n):
                    nc.tensor.matmul(
                        dummy_ps[0:128, 0:CC], zstrip[:, 0:128], zstrip[:],
                        start=True, stop=True, skip_group_check=True)

            for _rep in range(reps):
                # ---- x block-diagonal staging: two-half zero-fill so the
                # first scatters start early; scatters are DMA (partition
                # offsets are illegal for engine APs) ----
                xd = prod.tile([64, 128, CC], F16)   # [k=(j,di), col=(j,b), cc]
                nc.vector.tensor_copy(
                    xd[:, 0:64, :],
                    zstrip[:].unsqueeze(1).broadcast_to((64, 64, CC)))
                xz = xd[:, 64:128, :].bitcast(mybir.dt.uint32)
                nc.scalar.mul(xz, xz, 0.0)
                for j in range(J):
                    nc.sync.dma_start(
                        xd[8 * j: 8 * j + 8, 16 * j: 16 * j + 16, :],
                        xf_d[j])

                pe_warm(70)

                u = main.tile([128, CC, DO, O], F16)

                # ---- produce u_hat; s0 fold interleaved on PE; copies Act/DVE
                s0 = pss.tile([128, DO, O], F32, tag="s")
                for g in range(NB + PROD_LAG):
                    if g < NB:
                        ps = pp.tile([128, PROD_BATCH, 512], F32, tag="pp")
                        for i in range(PROD_BATCH):
                            cc = g * PROD_BATCH + i
                            nc.tensor.matmul(
                                ps[:, i, 0:OD], xd[:, :, cc], wt[:, cc, :, :],
                                start=True, stop=True,
                            )
                        sl = slice(g * PROD_BATCH, (g + 1) * PROD_BATCH)
                        src = ps[:, :, 0:OD].rearrange(
                            "p c (do o) -> p c do o", do=DO)
                        # alternate copies Act/DVE (GPSIMD cannot read PSUM)
                        if g % 2 == 0:
                            nc.scalar.copy(u[:, sl, :, :], src)
                        else:
                            nc.vector.tensor_copy(u[:, sl, :, :], src)
                    if g >= PROD_LAG:
                        gs = g - PROD_LAG
                        for i in range(PROD_BATCH):
                            cc = gs * PROD_BATCH + i
                            nc.tensor.matmul(
                                s0[:], d16s[:], u[:, cc, :, :],
                                start=(cc == 0), stop=(cc == CC - 1),
                            )

                v = main.tile([128, DO, O], F16)
                _squash_chain(nc, sq, s0, v, eps)

                b16 = main.tile([128, CC, O], F16)
                e = main.tile([128, CC, O], F32)
                inv = main.tile([128, O], F32)
                c16 = main.tile([128, CC, O], F16)

                for it in (1, 2):
                    final = it == 2
                    den = psd.tile([128, O], F32, tag="den")
                    # ---- agreement: b_ij (+)= sum_do u * v  (premult + tree)
                    for g in range(NG):
                        last = g == NG - 1
                        sl = slice(g * TREE_BATCH, (g + 1) * TREE_BATCH)
                        t = tp.tile([128, TREE_BATCH, DO, O], F16, tag="t")
                        v_b = v[:].unsqueeze(1).broadcast_to(
                            (128, TREE_BATCH, DO, O))
                        nc.vector.tensor_mul(t[:], u[:, sl, :, :], v_b)
                        l1 = l1p.tile([128, TREE_BATCH, 8, O], F16, tag="l1")
                        nc.vector.tensor_add(
                            l1[:], t[:, :, 0:8, :], t[:, :, 8:16, :])
                        l2 = l2p.tile([128, TREE_BATCH, 4, O], F16, tag="l2")
                        nc.vector.tensor_add(
                            l2[:], l1[:, :, 0:4, :], l1[:, :, 4:8, :])
                        # low tree levels on GPSIMD, except the last batch
                        # (DVE finishes it immediately -> short tail)
                        eng = nc.vector if last else nc.gpsimd
                        l3 = l3p.tile([128, TREE_BATCH, 2, O], F16, tag="l3")
                        eng.tensor_add(
                            l3[:], l2[:, :, 0:2, :], l2[:, :, 2:4, :])
                        if it == 1:
                            eng.tensor_add(
                                b16[:, sl, :], l3[:, :, 0, :], l3[:, :, 1, :])
                        else:
                            l4 = l4p.tile([128, TREE_BATCH, O], F16, tag="l4")
                            eng.tensor_add(l4[:], l3[:, :, 0, :], l3[:, :, 1, :])
                            eng.tensor_add(
                                b16[:, sl, :], b16[:, sl, :], l4[:])
                        # exp in f32 (no overflow, no max pass needed)
                        nc.scalar.activation(
                            e[:, sl, :], b16[:, sl, :],
                            mybir.ActivationFunctionType.Exp, bias=zero[:])
                        # softmax denominator folded on PE (idle here)
                        for i in range(TREE_BATCH):
                            cc = g * TREE_BATCH + i
                            nc.tensor.matmul(
                                den[:], d32[:], e[:, cc, :],
                                start=(cc == 0), stop=(cc == CC - 1),
                            )
                        # keep PE hot while it waits for the next exp
                        if not last:
                            pe_warm(12)

                    nc.vector.reciprocal(inv[:], den[:])

                    # ---- s = sum_r c*u: c16 on Pool, premult DVE, fold PE ----
                    sp_p = BC if final else 128
                    lhs = dout[:] if final else d16[:]
                    s_ps2 = pss.tile([sp_p, DO, O], F32, tag="s")
                    for g in range(NG):
                        sl = slice(g * TREE_BATCH, (g + 1) * TREE_BATCH)
                        inv_b = inv[:].unsqueeze(1).broadcast_to(
                            (128, TREE_BATCH, O))
                        nc.gpsimd.tensor_mul(c16[:, sl, :], e[:, sl, :], inv_b)
                        t = tp.tile([128, TREE_BATCH, DO, O], F16, tag="t")
                        c_b = c16[:, sl, :].unsqueeze(2).broadcast_to(
                            (128, TREE_BATCH, DO, O))
                        nc.vector.tensor_mul(t[:], u[:, sl, :, :], c_b)
                        for i in range(TREE_BATCH):
                            cc = g * TREE_BATCH + i
                            nc.tensor.matmul(
                                s_ps2[:], lhs[:, :sp_p], t[:, i, :, :],
                                start=(cc == 0), stop=(cc == CC - 1),
                            )
                    if not final:
                        _squash_chain(nc, sq, s_ps2, v, eps)
                    else:
                        v2 = main.tile([BC, DO, O], F32)
                        _squash_chain(nc, sq, s_ps2, v2, eps)
                        v2p = main.tile([BC, O, DO], F32)
                        nc.vector.tensor_copy(v2p[:], v2[:].transpose((0, 2, 1)))
                        nc.sync.dma_start(out_d[:], v2p[:])

    nc.compile()
    return nc


_CACHE = {}


def _get_nc():
    if "nc" not in _CACHE:
        _CACHE["nc"] = build_nc()
    return _CACHE["nc"]


def _prep_const():
    if "const" not in _CACHE:
        p = np.arange(128)
        dout = (p[:, None] % 16 == np.arange(BC)[None, :]).astype(np.float16)
        _CACHE["const"] = dout
    return _CACHE["const"]


def kernel(x: np.ndarray, W: np.ndarray) -> np.ndarray:
    x = np.asarray(x, dtype=np.float32)
    W = np.asarray(W, dtype=np.float32)
    nc = _get_nc()
    dout = _prep_const()
    W5 = np.ascontiguousarray(W.reshape(R, O, DO, DI))
    # wt[8j+di, cc, do, o] = W[8cc+j, o, do, di]
    wt = np.ascontiguousarray(
        W5.reshape(CC, J, O, DO, DI).transpose(1, 4, 0, 3, 2)
    ).reshape(64, CC, DO, O).astype(np.float16)
    in_maps = []
    for q in range(NCORES):
        xq = x[BC * q: BC * (q + 1)]             # [16, 1152, 8]
        # xf[j, di, b, cc] = xq[b, 8cc+j, di]
        xf = np.ascontiguousarray(
            xq.reshape(BC, CC, J, DI).transpose(2, 3, 0, 1)
        ).astype(np.float16)
        in_maps.append({"wt": wt, "xf": xf, "dout": dout})
    res = run_bass_kernel_spmd(nc, in_maps, core_ids=list(range(NCORES)))
    out = np.concatenate([res.results[q]["out"] for q in range(NCORES)], axis=0)
    return out.reshape(B, O, DO, 1).astype(np.float32)
